# revision 1
# baseline (speedup 1.0000x reference)
"""GQA attention kernel for 8 trn2 NeuronCores (tensor-parallel over heads).

Problem: B=1, S=2048, D=2048, NQ=32 q heads, NKV=8 kv heads, HD=64.
Core i handles q heads 4i..4i+3 and kv head i; out = sum of per-core partials.

Layout strategy (all transposed, zero P-matrix transposes):
  x^T tiles  built on device via PE transpose (fp32 has no DMA transpose)
  Q^T [128=2 heads x 64, S] per head-pair, K^T [64, S]   (proj matmuls)
  V   [S, 64+1] normal layout + ones column (softmax sums come free from PV)
  S^T block = K^T_slice.T @ Q^T  -> exp on ACT -> PV: V_ext.T @ expS^T
  row 64 of PV psum = softmax denominators; normalize via K=1 bcast matmul
  out-proj: lhsT = O^T directly (no transpose), partial written to DRAM
RMSNorm over head dim (= partitions) via ones-selector matmuls on PE.
"""

import os
import sys

sys.path.insert(0, "/opt/trn_rl_repo")

import numpy as np

S = 2048
D = 2048
HD = 64
NQ = 32
NKV = 8
P = 128
EPS = 1e-6
SCALE = 0.125  # 1/sqrt(HD)
N_CORES = 8

_CACHE = {}
LAST_RESULTS = None


def _build_nc():
    import concourse.bass as bass
    import concourse.tile as tile
    from concourse import bacc, mybir

    f32 = mybir.dt.float32
    nc = bacc.Bacc("TRN2", target_bir_lowering=False, debug=False)

    def dram_in(name, shape):
        return nc.dram_tensor(name, list(shape), f32, kind="ExternalInput").ap()

    io = {
        "x2d": dram_in("x2d", (S, D)),
        "wqa": dram_in("wqa", (P, 16, P)),
        "wqb": dram_in("wqb", (P, 16, P)),
        "wk": dram_in("wk", (P, 16, HD)),
        "wv": dram_in("wv", (P, 16, HD)),
        "wo": dram_in("wo", (P, 2, D)),
        "cos4": dram_in("cos4", (P, S)),
        "sin4s": dram_in("sin4s", (P, S)),
        "gq2": dram_in("gq2", (P, 1)),
        "gk": dram_in("gk", (HD, 1)),
        "maskz": dram_in("maskz", (P, 1024)),
        "ones2": dram_in("ones2", (P, 2)),
        "ones64": dram_in("ones64", (1, HD)),
        "rot2": dram_in("rot2", (P, P)),
        "ones65": dram_in("ones65", (HD + 1, HD)),
        "ident": dram_in("ident", (P, P)),
        "out": nc.dram_tensor("out", [S, D], f32, kind="ExternalOutput").ap(),
    }

    from contextlib import ExitStack

    with tile.TileContext(nc) as tc, ExitStack() as ctx:
        _emit(ctx, tc, io, bass, mybir)
    nc.compile()
    return nc


def _emit(ctx, tc, io, bass, mybir):
    nc = tc.nc
    f32 = mybir.dt.float32
    Exp = mybir.ActivationFunctionType.Exp
    Sqrt = mybir.ActivationFunctionType.Sqrt
    mult = mybir.AluOpType.mult

    cpool = ctx.enter_context(tc.tile_pool(name="consts", bufs=1))
    pers = ctx.enter_context(tc.tile_pool(name="persist", bufs=1))

    # ---- constants / weights into SBUF ----
    def cload(name, shape):
        t = cpool.tile(list(shape), f32, tag=name, name=name)
        nc.sync.dma_start(t[:], io[name][:])
        return t

    wqa = cload("wqa", (P, 16, P))
    wqb = cload("wqb", (P, 16, P))
    wk = cload("wk", (P, 16, HD))
    wv = cload("wv", (P, 16, HD))
    wo = cload("wo", (P, 2, D))
    cos4 = cload("cos4", (P, S))
    sin4s = cload("sin4s", (P, S))
    gq2 = cload("gq2", (P, 1))
    gk = cload("gk", (HD, 1))
    maskz = cload("maskz", (P, 1024))
    ones2 = cload("ones2", (P, 2))
    ones64 = cload("ones64", (1, HD))
    rot2 = cload("rot2", (P, P))
    ones65 = cload("ones65", (HD + 1, HD))
    ident = cload("ident", (P, P))

    # ---- persistent activations ----
    QT = [pers.tile([P, S], f32, tag=f"qt{t}", name=f"QT{t}") for t in range(2)]  # head pairs
    KT = pers.tile([P, S], f32, tag="kt")  # rows 64-127 = duplicate of 0-63
    V = pers.tile([P, 16, HD + 1], f32, tag="v")  # [seq128, kblock, hd+ones]
    OT = pers.tile([P, 2, S], f32, tag="ot")  # attn out transposed
    stdq = [pers.tile([2, S], f32, tag=f"stdq{t}", name=f"stdq{t}") for t in range(2)]
    stdk = pers.tile([1, S], f32, tag="stdk")

    nc.vector.memset(V[:, :, HD : HD + 1], 1.0)
    epsc = pers.tile([P, 1], f32, tag="epsc")
    nc.vector.memset(epsc[:], EPS)

    # ================= Phase 1: transpose x + projections =================
    with (
        tc.tile_pool(name="xraw", bufs=2) as xrp,
        tc.tile_pool(name="xt", bufs=1) as xtp,
        tc.tile_pool(name="sq", bufs=2) as sqp,
        tc.tile_pool(name="tpsum", bufs=3, space="PSUM") as tp,
        tc.tile_pool(name="ppsum", bufs=2, space="PSUM") as pp,
        tc.tile_pool(name="vpsum", bufs=2, space="PSUM") as vp,
        tc.tile_pool(name="sspsum", bufs=1, space="PSUM") as ssp,
    ):
        for sc in range(4):  # seq chunks of 512
            xT = xtp.tile([P, 16, 512], f32, tag="xt")
            for sb in range(4):  # 128-row blocks
                xraw = xrp.tile([P, D], f32, tag="xraw")
                r0 = sc * 512 + sb * P
                nc.sync.dma_start(xraw[:], io["x2d"][r0 : r0 + P, :])
                for kc in range(16):
                    pt = tp.tile([P, P], f32, tag="t")
                    nc.tensor.transpose(pt[:], xraw[:, kc * P : (kc + 1) * P], ident[:])
                    nc.any.tensor_copy(xT[:, kc, sb * P : (sb + 1) * P], pt[:])

            cs = slice(sc * 512, (sc + 1) * 512)

            def proj(lhsT_w, m, dst_copy, ss_dst=None, n_ones=None):
                ps = pp.tile([P, 512], f32, tag="p", name="ps")[:m]
                for kc in range(16):
                    nc.tensor.matmul(
                        ps, lhsT_w[:, kc, :], xT[:, kc, :],
                        start=(kc == 0), stop=(kc == 15),
                    )
                dst_copy(ps)
                if ss_dst is not None:
                    sq = sqp.tile([P, 512], f32, tag="sq", name="sq")[:m]
                    nc.scalar.activation(sq, ps, mybir.ActivationFunctionType.Square)
                    nm = ss_dst.shape[0]
                    ssps = ssp.tile([2, 512], f32, tag="ss", name="ssps")[:nm]
                    nc.tensor.matmul(ssps, n_ones, sq, start=True, stop=True)
                    # std = sqrt(mean + eps)
                    nc.scalar.activation(ss_dst, ssps, Sqrt, bias=epsc[:nm], scale=1.0 / HD)

            proj(wqa, P, lambda ps: nc.vector.tensor_copy(QT[0][:, cs], ps),
                 ss_dst=stdq[0][:, cs], n_ones=ones2[:, :])
            proj(wqb, P, lambda ps: nc.vector.tensor_copy(QT[1][:, cs], ps),
                 ss_dst=stdq[1][:, cs], n_ones=ones2[:, :])
            proj(wk, HD, lambda ps: nc.vector.tensor_copy(KT[0:HD, cs], ps),
                 ss_dst=stdk[:, cs], n_ones=ones2[:HD, 0:1])
            # V in normal layout: lhsT = x^T slice, rhs = wv
            for ms in range(4):
                pv = vp.tile([P, HD], f32, tag="v")
                for kc in range(16):
                    nc.tensor.matmul(
                        pv[:], xT[:, kc, ms * P : (ms + 1) * P], wv[:, kc, :],
                        start=(kc == 0), stop=(kc == 15),
                    )
                nc.any.tensor_copy(V[:, sc * 4 + ms, 0:HD], pv[:])

    # ================= Phase 2: RMSNorm + RoPE (in place) =================
    with (
        tc.tile_pool(name="rtmp", bufs=2) as rtp,
        tc.tile_pool(name="rsm", bufs=2) as rsp,
        tc.tile_pool(name="bcpsum", bufs=2, space="PSUM") as bcp,
        tc.tile_pool(name="swpsum", bufs=2, space="PSUM") as swp,
        tc.tile_pool(name="selp", bufs=1, space="PSUM") as selpp,
    ):
        # selector for head-pair bcast: sel2 [2, P] = ones2.T (via PE transpose)
        selps = selpp.tile([2, P], f32, tag="sel")
        nc.tensor.transpose(selps[:], ones2[:, :], ident[:])
        sel2 = rsp.tile([2, P], f32, tag="sel2")
        nc.vector.tensor_copy(sel2[:], selps[:])

        def norm_rope(T, std, g, sel, m):
            # T [m, S], heads on 64-row groups; std [nh, S]; all base partition 0
            nh = std.shape[0]
            rstd = rsp.tile([2, S], f32, tag="rstd", name="rstd")[:nh]
            nc.vector.reciprocal(rstd, std)
            tmpc = rtp.tile([P, S], f32, tag="tc", name="tmpc")[:m]
            for c in range(4):
                cs = slice(c * 512, (c + 1) * 512)
                bc = bcp.tile([P, 512], f32, tag="bc", name="bc")[:m]
                nc.tensor.matmul(bc, sel, rstd[:, cs], start=True, stop=True)
                # T = (T * g) * bcast(rstd)   in place
                nc.vector.scalar_tensor_tensor(T[:, cs], T[:, cs], g, bc, mult, mult)
            nc.vector.tensor_mul(tmpc, T, cos4[:m, :])
            for c in range(4):
                cs = slice(c * 512, (c + 1) * 512)
                sw = swp.tile([P, 512], f32, tag="sw", name="sw")[:m]
                nc.tensor.matmul(sw, rot2[:m, :m], T[:, cs], start=True, stop=True)
                # T_chunk = swap(T_normed) * sin4s  (tmpc added after loop)
                nc.vector.tensor_mul(T[:, cs], sw, sin4s[:m, cs])
            nc.vector.tensor_add(T, T, tmpc)

        norm_rope(QT[0][:, :], stdq[0][:, :], gq2[:, :], sel2[:, :], P)
        norm_rope(QT[1][:, :], stdq[1][:, :], gq2[:, :], sel2[:, :], P)
        norm_rope(KT[0:HD, :], stdk[:, :], gk[:, :], ones64[:, :], HD)
        # duplicate normed+roped K into partitions 64-127 (for odd-head scores)
        nc.sync.dma_start(KT[HD:P, :], KT[0:HD, :])

    # ================= Phase 3: attention + out-projection =================
    with (
        tc.tile_pool(name="exps", bufs=3) as ep,
        tc.tile_pool(name="bcs", bufs=2) as bcsp,
        tc.tile_pool(name="ov", bufs=2) as ovp,
        tc.tile_pool(name="spsum", bufs=3, space="PSUM") as sp,
        tc.tile_pool(name="opsum", bufs=2, space="PSUM") as op_,
        tc.tile_pool(name="bpsum", bufs=1, space="PSUM") as bp,
        tc.tile_pool(name="oppsum", bufs=2, space="PSUM") as opp,
    ):
        for qc in range(4):
            qs = slice(qc * 512, (qc + 1) * 512)
            for h in range(4):
                pair, poff = h // 2, (h % 2) * HD
                Q = QT[pair]
                nkb = 4 * qc + 4
                po = op_.tile([HD + 1, 512], f32, tag="o")

                def score_exp(kb):
                    ps = sp.tile([P, 512], f32, tag="s")
                    nc.tensor.matmul(
                        ps,
                        KT[poff : poff + HD, kb * P : (kb + 1) * P],
                        Q[poff : poff + HD, qs],
                        start=True, stop=True,
                    )
                    es = ep.tile([P, 512], f32, tag="e")
                    nc.scalar.activation(es, ps, Exp, scale=SCALE)
                    o = kb - 4 * qc
                    if o >= 0:
                        mz = maskz[:, 512 - o * P : 1024 - o * P]
                        nc.vector.tensor_mul(es, es, mz)
                    return es

                def pv(kb, es):
                    nc.tensor.matmul(
                        po, V[:, kb, :], es,
                        start=(kb == 0), stop=(kb == nkb - 1),
                    )

                prev = score_exp(0)
                for kb in range(1, nkb):
                    cur = score_exp(kb)
                    pv(kb - 1, prev)
                    prev = cur
                pv(nkb - 1, prev)

                # normalize: row HD of po holds the softmax denominators
                rec = bcsp.tile([HD + 1, 512], f32, tag="rec", name="rec")[HD : HD + 1]
                nc.vector.reciprocal(rec, po[HD : HD + 1, :])
                bc = bp.tile([HD, 512], f32, tag="b")
                nc.tensor.matmul(bc, ones65[HD : HD + 1, :], rec, start=True, stop=True)
                bcs = bcsp.tile([HD, 512], f32, tag="bcs")
                nc.vector.tensor_copy(bcs, bc)
                if poff == 0:
                    nc.vector.tensor_mul(OT[0:HD, pair, qs], po[0:HD, :], bcs)
                else:
                    stg = bcsp.tile([HD, 512], f32, tag="stg")
                    nc.vector.tensor_mul(stg, po[0:HD, :], bcs)
                    nc.sync.dma_start(OT[HD:P, pair, qs], stg[:])

            # out-projection for this q chunk (all 4 heads now done)
            for ms in range(4):
                sl = slice(qc * 512 + ms * P, qc * 512 + (ms + 1) * P)
                for dc in range(4):
                    pso = opp.tile([P, 512], f32, tag="op")
                    for kc in range(2):
                        nc.tensor.matmul(
                            pso, OT[:, kc, sl], wo[:, kc, dc * 512 : (dc + 1) * 512],
                            start=(kc == 0), stop=(kc == 1),
                        )
                    ov = ovp.tile([P, 512], f32, tag="ov")
                    nc.vector.tensor_copy(ov[:], pso[:])
                    nc.sync.dma_start(io["out"][sl, dc * 512 : (dc + 1) * 512], ov[:])


def _prep_core_inputs(i, x, cos, sin, g_q, g_k, Wq, Wk, Wv, Wo):
    c0 = i * 4 * HD
    k0 = i * HD
    wqa = np.ascontiguousarray(
        Wq[:, c0 : c0 + P].reshape(16, P, P).transpose(1, 0, 2))
    wqb = np.ascontiguousarray(
        Wq[:, c0 + P : c0 + 2 * P].reshape(16, P, P).transpose(1, 0, 2))
    wk = np.ascontiguousarray(
        Wk[:, k0 : k0 + HD].reshape(16, P, HD).transpose(1, 0, 2))
    wv = np.ascontiguousarray(
        Wv[:, k0 : k0 + HD].reshape(16, P, HD).transpose(1, 0, 2))
    wo = np.ascontiguousarray(
        Wo[c0 : c0 + 2 * P, :].reshape(2, P, D).transpose(1, 0, 2))
    cosT = cos.T.astype(np.float32)  # [32, S]
    sinT = sin.T.astype(np.float32)
    cos4 = np.tile(cosT, (4, 1))
    sin4s = np.concatenate([-sinT, sinT, -sinT, sinT], axis=0)
    gq2 = np.tile(g_q, 2)[:, None].astype(np.float32)
    gk = g_k[:, None].astype(np.float32)
    tri = np.triu(np.ones((P, P), dtype=np.float32))  # [k within blk, q within blk]
    mask0 = np.concatenate([tri, np.ones((P, 384), dtype=np.float32)], axis=1)
    maskz = np.concatenate([np.zeros((P, 512), dtype=np.float32), mask0], axis=1)
    ones2 = np.zeros((P, 2), dtype=np.float32)
    ones2[:HD, 0] = 1.0
    ones2[HD:, 1] = 1.0
    r64 = np.roll(np.eye(HD, dtype=np.float32), 32, axis=0)
    rot2 = np.zeros((P, P), dtype=np.float32)
    rot2[:HD, :HD] = r64
    rot2[HD:, HD:] = r64
    return {
        "x2d": np.ascontiguousarray(x.reshape(S, D)),
        "wqa": wqa, "wqb": wqb, "wk": wk, "wv": wv, "wo": wo,
        "cos4": np.ascontiguousarray(cos4), "sin4s": np.ascontiguousarray(sin4s),
        "gq2": gq2, "gk": gk, "maskz": maskz, "ones2": ones2,
        "ones64": np.ones((1, HD), dtype=np.float32),
        "rot2": rot2,
        "ones65": np.ones((HD + 1, HD), dtype=np.float32),
        "ident": np.eye(P, dtype=np.float32),
    }


def kernel(x, cos, sin, g_q, g_k, Wq, Wk, Wv, Wo):
    global LAST_RESULTS
    from concourse.bass_utils import run_bass_kernel_spmd

    if "nc" not in _CACHE:
        _CACHE["nc"] = _build_nc()
    nc = _CACHE["nc"]

    args = [np.asarray(a, dtype=np.float32) for a in
            (x, cos, sin, g_q, g_k, Wq, Wk, Wv, Wo)]
    in_maps = [_prep_core_inputs(i, *args) for i in range(N_CORES)]
    trace = bool(os.environ.get("BASS_TRACE"))
    res = run_bass_kernel_spmd(nc, in_maps, list(range(N_CORES)), trace=trace)
    LAST_RESULTS = res
    out = np.zeros((S, D), dtype=np.float32)
    for r in res.results:
        out += r["out"]
    return out.reshape(1, S, D)



# revision 4
# speedup vs baseline: 3.1779x; 3.1779x over previous
"""GQA attention kernel for 8 trn2 NeuronCores (tensor-parallel over heads).

Problem: B=1, S=2048, D=2048, NQ=32 q heads, NKV=8 kv heads, HD=64.
Core i handles q heads 4i..4i+3 and kv head i; out = sum of per-core partials.

v2: all matmuls in bf16 (fp32 runs at 1/4 rate on the PE), x^T prepared on
host (kills 256 on-device PE transposes), paired-head score matmuls issued
to disjoint PE row groups (K=64 each -> concurrent), exp batched over
[128, 2x512] PSUM groups, bf16 partial outputs summed on host.

Layout (all transposed, zero on-device transposes):
  xT  [128, 4(sc), 16(kc), 512] bf16   host-pretransposed activations
  Q^T [128=2 heads x 64, S] per head pair, K^T [128, S] (dup for row pairing)
  V   [S, 16, 64+1] bf16 + ones column (softmax sums come free from PV)
  S^T block pair = KT.T @ QT (two row-group matmuls) -> one exp on ACT
  PV: V_ext.T @ expS^T ; row 64 = softmax denominators
  out-proj: lhsT = O^T directly, bf16 partial written to DRAM
RMSNorm over head dim (= partitions) via ones-selector matmuls on PE.
"""

import os
import sys

sys.path.insert(0, "/opt/trn_rl_repo")

import numpy as np
import ml_dtypes

BF16 = ml_dtypes.bfloat16

S = 2048
D = 2048
HD = 64
NQ = 32
NKV = 8
P = 128
EPS = 1e-6
SCALE = 0.125  # 1/sqrt(HD)
N_CORES = 8

_CACHE = {}
LAST_RESULTS = None


def _build_nc():
    import concourse.bass as bass
    import concourse.tile as tile
    from concourse import bacc, mybir

    f32 = mybir.dt.float32
    bf = mybir.dt.bfloat16
    nc = bacc.Bacc("TRN2", target_bir_lowering=False, debug=False)

    def dram_in(name, shape, dt):
        return nc.dram_tensor(name, list(shape), dt, kind="ExternalInput").ap()

    io = {
        "xt4": dram_in("xt4", (P, 4, 16, 512), bf),
        "wqa": dram_in("wqa", (P, 16, P), bf),
        "wqb": dram_in("wqb", (P, 16, P), bf),
        "wk": dram_in("wk", (P, 16, HD), bf),
        "wv": dram_in("wv", (P, 16, HD), bf),
        "wo": dram_in("wo", (P, 2, D), bf),
        "cos4": dram_in("cos4", (P, S), bf),
        "sin4s": dram_in("sin4s", (P, S), bf),
        "gq2": dram_in("gq2", (P, 1), f32),
        "gk": dram_in("gk", (HD, 1), f32),
        "maskp": dram_in("maskp", (P, 2, 1024), bf),
        "ones2": dram_in("ones2", (P, 2), bf),
        "sel2": dram_in("sel2", (2, P), bf),
        "onesk": dram_in("onesk", (1, HD), bf),
        "rot2": dram_in("rot2", (P, P), bf),
        "out": nc.dram_tensor("out", [S, D], bf, kind="ExternalOutput").ap(),
    }

    from contextlib import ExitStack

    with tile.TileContext(nc) as tc, ExitStack() as ctx:
        _emit(ctx, tc, io, bass, mybir)
    nc.compile()
    return nc


def _emit(ctx, tc, io, bass, mybir):
    nc = tc.nc
    f32 = mybir.dt.float32
    bf = mybir.dt.bfloat16
    Exp = mybir.ActivationFunctionType.Exp
    Sqrt = mybir.ActivationFunctionType.Sqrt
    Square = mybir.ActivationFunctionType.Square
    Copy = mybir.ActivationFunctionType.Copy
    mult = mybir.AluOpType.mult
    add = mybir.AluOpType.add

    cpool = ctx.enter_context(tc.tile_pool(name="consts", bufs=1))
    pers = ctx.enter_context(tc.tile_pool(name="persist", bufs=1))

    # ---- constants / weights into SBUF ----
    def cload(name, shape, dt):
        t = cpool.tile(list(shape), dt, tag=name, name=name)
        nc.sync.dma_start(t[:], io[name][:])
        return t

    wqa = cload("wqa", (P, 16, P), bf)
    wqb = cload("wqb", (P, 16, P), bf)
    wk = cload("wk", (P, 16, HD), bf)
    wv = cload("wv", (P, 16, HD), bf)
    wo = cload("wo", (P, 2, D), bf)
    cos4 = cload("cos4", (P, S), bf)
    sin4s = cload("sin4s", (P, S), bf)
    gq2 = cload("gq2", (P, 1), f32)
    gk = cload("gk", (HD, 1), f32)
    maskp = cload("maskp", (P, 2, 1024), bf)
    ones2 = cload("ones2", (P, 2), bf)
    sel2 = cload("sel2", (2, P), bf)
    onesk = cload("onesk", (1, HD), bf)
    rot2 = cload("rot2", (P, P), bf)

    # ---- persistent activations ----
    QT = [pers.tile([P, S], bf, tag=f"qt{t}", name=f"QT{t}") for t in range(2)]
    KT = pers.tile([P, S], bf, tag="kt")  # rows 64-127 = duplicate of 0-63
    V = pers.tile([P, 16, HD + 1], bf, tag="v")  # [seq128, kblock, hd+ones]
    OT = pers.tile([P, 2, S], bf, tag="ot")  # attn out transposed

    nc.vector.memset(V[:, :, HD : HD + 1], 1.0)
    epsc = pers.tile([P, 1], f32, tag="epsc")
    nc.vector.memset(epsc[:], EPS)

    # ============ Phase 1+2: projections + RMSNorm + RoPE ============
    with (
        tc.tile_pool(name="xin", bufs=2) as xip,
        tc.tile_pool(name="sq", bufs=2) as sqp,
        tc.tile_pool(name="traw", bufs=2) as trp,
        tc.tile_pool(name="stdv", bufs=2) as stdp,
        tc.tile_pool(name="rstd", bufs=2) as rsp,
        tc.tile_pool(name="tnorm", bufs=2) as tnp,
        tc.tile_pool(name="tcos", bufs=2) as tcp,
        tc.tile_pool(name="tsin", bufs=2) as t1p,
        tc.tile_pool(name="ppsum", bufs=2, space="PSUM") as pp,
        tc.tile_pool(name="sspsum", bufs=1, space="PSUM") as ssp,
        tc.tile_pool(name="bcpsum", bufs=1, space="PSUM") as bcp,
        tc.tile_pool(name="swpsum", bufs=1, space="PSUM") as swp,
        tc.tile_pool(name="vpsum", bufs=2, space="PSUM") as vp,
    ):
        for sc in range(4):
            cs = slice(sc * 512, (sc + 1) * 512)
            xs = xip.tile([P, 16, 512], bf, tag="xs", name="xs")
            nc.sync.dma_start(xs[:], io["xt4"][:, sc, :, :])

            def proj_norm_rope(lhsT_w, m, g, sel, nh, dst):
                # projection into PSUM
                ps = pp.tile([P, 512], f32, tag="p", name="ps")[:m]
                for kc in range(16):
                    nc.tensor.matmul(
                        ps, lhsT_w[:, kc, :], xs[:, kc, :],
                        start=(kc == 0), stop=(kc == 15),
                    )
                # rms stats: sq = ps^2 (bf16), column sums via ones matmul
                sq = sqp.tile([P, 512], bf, tag="sq", name="sq")[:m]
                nc.scalar.activation(sq, ps, Square)
                traw = trp.tile([P, 512], f32, tag="tr", name="traw")[:m]
                nc.scalar.activation(traw, ps, Copy)
                ssps = ssp.tile([2, 512], f32, tag="ss", name="ssps")[:nh]
                nc.tensor.matmul(ssps, sel[:, :nh] if nh == 2 else sel[:, 0:1],
                                 sq, start=True, stop=True)
                std = stdp.tile([2, 512], f32, tag="std", name="std")[:nh]
                nc.scalar.activation(std, ssps, Sqrt, bias=epsc[:nh], scale=1.0 / HD)
                rstd = rsp.tile([2, 512], bf, tag="rstd", name="rstd")[:nh]
                with nc.allow_low_precision(reason="bf16 rstd feeds bf16 matmul"):
                    nc.vector.reciprocal(rstd, std)
                # broadcast 1/std across the 64-partition head groups
                bc = bcp.tile([P, 512], f32, tag="bc", name="bc")[:m]
                nc.tensor.matmul(bc, sel2[:nh, :m] if nh == 2 else onesk[:, :m],
                                 rstd, start=True, stop=True)
                # normalize: tn = (traw * g) * bc   (bf16 out)
                tn = tnp.tile([P, 512], bf, tag="tn", name="tn")[:m]
                nc.vector.scalar_tensor_tensor(tn, traw, g, bc, mult, mult)
                # rope: dst = tn*cos + swap(tn)*sin
                tmpc = tcp.tile([P, 512], bf, tag="tc", name="tmpc")[:m]
                nc.vector.tensor_mul(tmpc, tn, cos4[:m, cs])
                sw = swp.tile([P, 512], f32, tag="sw", name="sw")[:m]
                nc.tensor.matmul(sw, rot2[:m, :m], tn, start=True, stop=True)
                t1 = t1p.tile([P, 512], bf, tag="t1", name="t1")[:m]
                nc.vector.tensor_mul(t1, sw, sin4s[:m, cs])
                nc.vector.tensor_add(dst, t1, tmpc)

            proj_norm_rope(wqa, P, gq2[:, :], ones2, 2, QT[0][:, cs])
            proj_norm_rope(wqb, P, gq2[:, :], ones2, 2, QT[1][:, cs])
            proj_norm_rope(wk, HD, gk[:, :], ones2[:HD, :], 1, KT[0:HD, cs])
            # duplicate normed+roped K into partitions 64-127 (row pairing)
            nc.sync.dma_start(KT[HD:P, cs], KT[0:HD, cs])
            # V in normal layout: lhsT = x^T slice, rhs = wv
            for ms in range(4):
                pv = vp.tile([P, HD], f32, tag="v", name="pv")
                for kc in range(16):
                    nc.tensor.matmul(
                        pv[:], xs[:, kc, ms * P : (ms + 1) * P], wv[:, kc, :],
                        start=(kc == 0), stop=(kc == 15),
                    )
                nc.vector.tensor_copy(V[:, sc * 4 + ms, 0:HD], pv[:])

    # ============ Phase 3: attention + out-projection ============
    with (
        tc.tile_pool(name="exps", bufs=3) as ep,
        tc.tile_pool(name="recs", bufs=2) as rcp,
        tc.tile_pool(name="bcs", bufs=2) as bcsp,
        tc.tile_pool(name="stg", bufs=2) as stgp,
        tc.tile_pool(name="ov", bufs=2) as ovp,
        tc.tile_pool(name="spsum", bufs=2, space="PSUM") as sp,
        tc.tile_pool(name="opsum", bufs=2, space="PSUM") as op_,
        tc.tile_pool(name="bpsum", bufs=1, space="PSUM") as bp,
        tc.tile_pool(name="oppsum", bufs=1, space="PSUM") as opp,
    ):
        for qc in range(4):
            qs = slice(qc * 512, (qc + 1) * 512)
            nkb = 4 * qc + 4
            for pair in range(2):
                Q = QT[pair]
                po = [op_.tile([HD + 1, 512], f32, tag="o", name="po")
                      for _ in range(2)]

                def score_exp(kb):
                    ps2 = sp.tile([P, 2, 512], f32, tag="s", name="ps2")
                    kbs = slice(kb * P, (kb + 1) * P)
                    nc.tensor.matmul(ps2[:, 0, :], KT[0:HD, kbs],
                                     Q[0:HD, qs], start=True, stop=True,
                                     tile_position=(0, 0))
                    nc.tensor.matmul(ps2[:, 1, :], KT[HD:P, kbs],
                                     Q[HD:P, qs], start=True, stop=True,
                                     tile_position=(HD, 0))
                    es2 = ep.tile([P, 2, 512], bf, tag="e", name="es2")
                    nc.scalar.activation(es2[:], ps2[:], Exp, scale=SCALE)
                    o = kb - 4 * qc
                    if o >= 0:
                        mz = maskp[:, :, 512 - o * P : 1024 - o * P]
                        nc.vector.tensor_mul(es2[:], es2[:], mz)
                    return es2

                def pv_acc(kb, es2):
                    st = (kb == 0)
                    sp_ = (kb == nkb - 1)
                    for j in range(2):
                        nc.tensor.matmul(po[j], V[:, kb, :], es2[:, j, :],
                                         start=st, stop=sp_)

                prev = score_exp(0)
                for kb in range(1, nkb):
                    cur = score_exp(kb)
                    pv_acc(kb - 1, prev)
                    prev = cur
                pv_acc(nkb - 1, prev)

                # normalize: row HD of po holds the softmax denominators
                for j in range(2):
                    rec = rcp.tile([1, 512], bf, tag="rec", name="rec")
                    with nc.allow_low_precision(reason="bf16 softmax denom"):
                        nc.vector.reciprocal(rec, po[j][HD : HD + 1, :])
                    bcd = bp.tile([HD, 512], f32, tag="b", name="bcd")
                    nc.tensor.matmul(bcd, onesk[:, :], rec, start=True,
                                     stop=True)
                    bcs = bcsp.tile([HD, 512], bf, tag="bcs", name="bcs")
                    nc.vector.tensor_copy(bcs, bcd)
                    if j == 0:
                        nc.vector.tensor_mul(OT[0:HD, pair, qs],
                                             po[j][0:HD, :], bcs)
                    else:
                        stg = stgp.tile([HD, 512], bf, tag="stg", name="stg")
                        nc.vector.tensor_mul(stg, po[j][0:HD, :], bcs)
                        nc.sync.dma_start(OT[HD:P, pair, qs], stg[:])

            # out-projection for this q chunk (all 4 heads now done)
            for ms in range(4):
                sl = slice(qc * 512 + ms * P, qc * 512 + (ms + 1) * P)
                for dc in range(4):
                    pso = opp.tile([P, 512], f32, tag="op", name="pso")
                    for kc in range(2):
                        nc.tensor.matmul(
                            pso, OT[:, kc, sl], wo[:, kc, dc * 512 : (dc + 1) * 512],
                            start=(kc == 0), stop=(kc == 1),
                        )
                    ov = ovp.tile([P, 512], bf, tag="ov", name="ov")
                    nc.vector.tensor_copy(ov[:], pso[:])
                    nc.sync.dma_start(io["out"][sl, dc * 512 : (dc + 1) * 512], ov[:])


def _prep_core_inputs(i, x, cos, sin, g_q, g_k, Wq, Wk, Wv, Wo):
    c0 = i * 4 * HD
    k0 = i * HD

    def b(a):
        return np.ascontiguousarray(a.astype(BF16))

    x2d = x.reshape(S, D)
    # xt4[p, sc, kc, j] = x[sc*512+j, kc*128+p]
    xt4 = b(x2d.T.reshape(16, P, 4, 512).transpose(1, 2, 0, 3))
    wqa = b(Wq[:, c0 : c0 + P].reshape(16, P, P).transpose(1, 0, 2))
    wqb = b(Wq[:, c0 + P : c0 + 2 * P].reshape(16, P, P).transpose(1, 0, 2))
    wk = b(Wk[:, k0 : k0 + HD].reshape(16, P, HD).transpose(1, 0, 2))
    wv = b(Wv[:, k0 : k0 + HD].reshape(16, P, HD).transpose(1, 0, 2))
    wo = b(Wo[c0 : c0 + 2 * P, :].reshape(2, P, D).transpose(1, 0, 2))
    cosT = cos.T.astype(np.float32)  # [32, S]
    sinT = sin.T.astype(np.float32)
    cos4 = b(np.tile(cosT, (4, 1)))
    sin4s = b(np.concatenate([-sinT, sinT, -sinT, sinT], axis=0))
    gq2 = np.tile(g_q, 2)[:, None].astype(np.float32)
    gk = g_k[:, None].astype(np.float32)
    tri = np.triu(np.ones((P, P), dtype=np.float32))  # [k within blk, q within blk]
    mask0 = np.concatenate([tri, np.ones((P, 384), dtype=np.float32)], axis=1)
    maskz = np.concatenate([np.zeros((P, 512), dtype=np.float32), mask0], axis=1)
    maskp = b(np.stack([maskz, maskz], axis=1))  # [128, 2, 1024]
    ones2 = np.zeros((P, 2), dtype=np.float32)
    ones2[:HD, 0] = 1.0
    ones2[HD:, 1] = 1.0
    r64 = np.roll(np.eye(HD, dtype=np.float32), 32, axis=0)
    rot2 = np.zeros((P, P), dtype=np.float32)
    rot2[:HD, :HD] = r64
    rot2[HD:, HD:] = r64
    return {
        "xt4": xt4,
        "wqa": wqa, "wqb": wqb, "wk": wk, "wv": wv, "wo": wo,
        "cos4": cos4, "sin4s": sin4s,
        "gq2": gq2, "gk": gk, "maskp": maskp,
        "ones2": b(ones2), "sel2": b(ones2.T),
        "onesk": np.ones((1, HD), dtype=BF16),
        "rot2": b(rot2),
    }


def kernel(x, cos, sin, g_q, g_k, Wq, Wk, Wv, Wo):
    global LAST_RESULTS
    from concourse.bass_utils import run_bass_kernel_spmd

    if "nc" not in _CACHE:
        _CACHE["nc"] = _build_nc()
    nc = _CACHE["nc"]

    args = [np.asarray(a, dtype=np.float32) for a in
            (x, cos, sin, g_q, g_k, Wq, Wk, Wv, Wo)]
    in_maps = [_prep_core_inputs(i, *args) for i in range(N_CORES)]
    trace = bool(os.environ.get("BASS_TRACE"))
    res = run_bass_kernel_spmd(nc, in_maps, list(range(N_CORES)), trace=trace)
    LAST_RESULTS = res
    out = np.zeros((S, D), dtype=np.float32)
    for r in res.results:
        out += np.asarray(r["out"], dtype=np.float32)
    return out.reshape(1, S, D)


# revision 23
# speedup vs baseline: 4.4918x; 1.4134x over previous
"""GQA attention kernel for 8 trn2 NeuronCores (tensor-parallel over heads).

Problem: B=1, S=2048, D=2048, NQ=32 q heads, NKV=8 kv heads, HD=64.
Core i handles q heads 4i..4i+3 and kv head i; out = sum of per-core partials.

v2: all matmuls in bf16 (fp32 runs at 1/4 rate on the PE), x^T prepared on
host (kills 256 on-device PE transposes), paired-head score matmuls issued
to disjoint PE row groups (K=64 each -> concurrent), exp batched over
[128, 2x512] PSUM groups, bf16 partial outputs summed on host.

Layout (all transposed, zero on-device transposes):
  xT  [128, 4(sc), 16(kc), 512] bf16   host-pretransposed activations
  Q^T [128=2 heads x 64, S] per head pair, K^T [128, S] (dup for row pairing)
  V   [S, 16, 64+1] bf16 + ones column (softmax sums come free from PV)
  S^T block pair = KT.T @ QT (two row-group matmuls) -> one exp on ACT
  PV: V_ext.T @ expS^T ; row 64 = softmax denominators
  out-proj: lhsT = O^T directly, bf16 partial written to DRAM
RMSNorm over head dim (= partitions) via ones-selector matmuls on PE.
"""

import os
import sys

sys.path.insert(0, "/opt/trn_rl_repo")

import numpy as np
import ml_dtypes

BF16 = ml_dtypes.bfloat16

S = 2048
D = 2048
HD = 64
NQ = 32
NKV = 8
P = 128
EPS = 1e-6
SCALE = 0.125  # 1/sqrt(HD)
N_CORES = 8

_CACHE = {}
LAST_RESULTS = None


def _build_nc():
    import concourse.bass as bass
    import concourse.tile as tile
    from concourse import bacc, mybir

    f32 = mybir.dt.float32
    bf = mybir.dt.bfloat16
    nc = bacc.Bacc("TRN2", target_bir_lowering=False, debug=False)

    def dram_in(name, shape, dt):
        return nc.dram_tensor(name, list(shape), dt, kind="ExternalInput").ap()

    io = {
        "xt4": dram_in("xt4", (P, 4, 16, 512), bf),
        "wqa": dram_in("wqa", (P, 16, P), bf),
        "wqb": dram_in("wqb", (P, 16, P), bf),
        "wk": dram_in("wk", (P, 16, HD), bf),
        "wv": dram_in("wv", (P, 16, HD), bf),
        "wo": dram_in("wo", (P, 2, D), bf),
        "cos4": dram_in("cos4", (P, S), bf),
        "sin4s": dram_in("sin4s", (P, S), bf),
        "gq2": dram_in("gq2", (P, 1), f32),
        "gk": dram_in("gk", (HD, 1), f32),
        "maskp": dram_in("maskp", (P, 2, 1024), bf),
        "ones65": dram_in("ones65", (P, HD + 1), bf),
        "rot2": dram_in("rot2", (P, P), bf),
        "out": nc.dram_tensor("out", [S, D], bf, kind="ExternalOutput").ap(),
    }

    from contextlib import ExitStack

    with tile.TileContext(nc) as tc, ExitStack() as ctx:
        _emit(ctx, tc, io, bass, mybir)
    nc.compile()
    return nc


def _emit(ctx, tc, io, bass, mybir):
    nc = tc.nc
    f32 = mybir.dt.float32
    bf = mybir.dt.bfloat16
    Exp = mybir.ActivationFunctionType.Exp
    Sqrt = mybir.ActivationFunctionType.Sqrt
    Square = mybir.ActivationFunctionType.Square
    Copy = mybir.ActivationFunctionType.Copy
    mult = mybir.AluOpType.mult
    add = mybir.AluOpType.add

    cpool = ctx.enter_context(tc.tile_pool(name="consts", bufs=1))
    pers = ctx.enter_context(tc.tile_pool(name="persist", bufs=1))

    # ---- constants / weights into SBUF ----
    def cload(name, shape, dt):
        t = cpool.tile(list(shape), dt, tag=name, name=name)
        nc.sync.dma_start(t[:], io[name][:])
        return t

    wqa = cload("wqa", (P, 16, P), bf)
    wqb = cload("wqb", (P, 16, P), bf)
    wk = cload("wk", (P, 16, HD), bf)
    wv = cload("wv", (P, 16, HD), bf)
    wo = cload("wo", (P, 2, D), bf)
    cos4 = cload("cos4", (P, S), bf)
    sin4s = cload("sin4s", (P, S), bf)
    gq2 = cload("gq2", (P, 1), f32)
    gk = cload("gk", (HD, 1), f32)
    maskp = cload("maskp", (P, 2, 1024), bf)
    ones65 = cload("ones65", (P, HD + 1), bf)
    rot2 = cload("rot2", (P, P), bf)

    # ---- persistent activations ----
    QT = [pers.tile([P, S], bf, tag=f"qt{t}", name=f"QT{t}") for t in range(2)]
    KT = pers.tile([P, S], bf, tag="kt")  # rows 64-127 = duplicate of 0-63
    V = pers.tile([P, 16, HD + 1], bf, tag="v")  # [seq128, kblock, hd+ones]
    OT = pers.tile([P, 2, S], bf, tag="ot")  # attn out transposed

    nc.vector.memset(V[:, :, HD : HD + 1], 1.0)
    epsc = pers.tile([P, 1], f32, tag="epsc")
    nc.vector.memset(epsc[:], EPS)

    # ============ Phase 1+2: projections + RMSNorm + RoPE ============
    with (
        tc.tile_pool(name="xin", bufs=2) as xip,
        tc.tile_pool(name="sq", bufs=2) as sqp,
        tc.tile_pool(name="stdv", bufs=2) as stdp,
        tc.tile_pool(name="rstd", bufs=2) as rsp,
        tc.tile_pool(name="bcast", bufs=2) as bcp,
        tc.tile_pool(name="tnorm", bufs=2) as tnp,
        tc.tile_pool(name="tcos", bufs=2) as tcp,
        tc.tile_pool(name="tsin", bufs=2) as t1p,
        tc.tile_pool(name="ppsum", bufs=3, space="PSUM") as pp,
        tc.tile_pool(name="sspsum", bufs=1, space="PSUM") as ssp,
        tc.tile_pool(name="swpsum", bufs=2, space="PSUM") as swp,
        tc.tile_pool(name="vpsum", bufs=2, space="PSUM") as vp,
    ):
        for sc in range(4):
            cs = slice(sc * 512, (sc + 1) * 512)
            xs = xip.tile([P, 16, 512], bf, tag="xs", name="xs")
            nc.sync.dma_start(xs[:], io["xt4"][:, sc, :, :])

            def proj_norm_rope(lhsT_w, m, g, sel, nh, dst):
                # projection into PSUM
                ps = pp.tile([P, 512], f32, tag="p", name="ps")[:m]
                for kc in range(16):
                    nc.tensor.matmul(
                        ps, lhsT_w[:, kc, :], xs[:, kc, :],
                        start=(kc == 0), stop=(kc == 15),
                    )
                # rms stats: sq = ps^2 (bf16); head sums land at partitions
                # 0 and 64 (selector cols 0/64) so all slices are p0/p64
                sq = sqp.tile([P, 512], bf, tag="sq", name="sq")[:m]
                nc.scalar.activation(sq, ps, Square)
                nss = HD + 1 if nh == 2 else 1
                ssps = ssp.tile([HD + 1, 512], f32, tag="ss", name="ssps")[:nss]
                nc.tensor.matmul(ssps, sel[:m, :nss], sq, start=True, stop=True)
                std = stdp.tile([HD + 1, 512], f32, tag="std", name="std")[:nss]
                nc.scalar.activation(std, ssps, Sqrt, bias=epsc[:nss],
                                     scale=1.0 / HD)
                # gpsimd broadcast can only write partition-0-based tiles;
                # the upper head group goes via a staging tile + SBUF DMA
                # approx_fast and partition_broadcast both need partition-0
                # sources on HW; relocate the p64 row via a DVE copy first
                bc = bcp.tile([P, 512], f32, tag="bc", name="bc")[:m]
                for h in range(nh):
                    rstd = rsp.tile([1, 512], f32, tag=f"rstd{h}",
                                    name=f"rstd{h}")
                    if h == 0:
                        nc.vector.reciprocal_approx_fast(rstd, std[0:1, :])
                        nc.gpsimd.partition_broadcast(bc[0:HD, :], rstd)
                    else:
                        stdc = rsp.tile([1, 512], f32, tag="stdc", name="stdc")
                        nc.vector.tensor_copy(stdc, std[HD : HD + 1, :])
                        nc.vector.reciprocal_approx_fast(rstd, stdc)
                        bch = bcp.tile([HD, 512], f32, tag="bch", name="bch")
                        nc.gpsimd.partition_broadcast(bch, rstd)
                        nc.sync.dma_start(bc[HD:P, :], bch[:])
                # normalize: tn = (ps * g) * bc   (bf16 out)
                tn = tnp.tile([P, 512], bf, tag="tn", name="tn")[:m]
                nc.vector.scalar_tensor_tensor(tn, ps, g, bc, mult, mult)
                # rope: dst = tn*cos + swap(tn)*sin
                tmpc = tcp.tile([P, 512], bf, tag="tc", name="tmpc")[:m]
                nc.vector.tensor_mul(tmpc, tn, cos4[:m, cs])
                sw = swp.tile([P, 512], f32, tag="sw", name="sw")[:m]
                nc.tensor.matmul(sw, rot2[:m, :m], tn, start=True, stop=True)
                t1 = t1p.tile([P, 512], bf, tag="t1", name="t1")[:m]
                nc.vector.tensor_mul(t1, sw, sin4s[:m, cs])
                nc.vector.tensor_add(dst, t1, tmpc)

            proj_norm_rope(wqa, P, gq2[:, :], ones65, 2, QT[0][:, cs])
            proj_norm_rope(wqb, P, gq2[:, :], ones65, 2, QT[1][:, cs])
            proj_norm_rope(wk, HD, gk[:, :], ones65, 1, KT[0:HD, cs])
            # duplicate normed+roped K into partitions 64-127 (row pairing)
            nc.sync.dma_start(KT[HD:P, cs], KT[0:HD, cs])
            # V in normal layout: lhsT = x^T slice, rhs = wv
            for ms in range(4):
                pv = vp.tile([P, HD], f32, tag="v", name="pv")
                for kc in range(16):
                    nc.tensor.matmul(
                        pv[:], xs[:, kc, ms * P : (ms + 1) * P], wv[:, kc, :],
                        start=(kc == 0), stop=(kc == 15),
                    )
                nc.vector.tensor_copy(V[:, sc * 4 + ms, 0:HD], pv[:])

    # ============ Phase 3: attention + out-projection ============
    with (
        tc.tile_pool(name="exps", bufs=3) as ep,
        tc.tile_pool(name="recs", bufs=2) as rcp,
        tc.tile_pool(name="bcs", bufs=2) as bcsp,
        tc.tile_pool(name="stg", bufs=2) as stgp,
        tc.tile_pool(name="ov", bufs=2) as ovp,
        tc.tile_pool(name="spsum", bufs=2, space="PSUM") as sp,
        tc.tile_pool(name="opsum", bufs=2, space="PSUM") as op_,
        tc.tile_pool(name="oppsum", bufs=2, space="PSUM") as opp,
    ):
        def out_proj(qc):
            # out-projection for q chunk qc (emitted one chunk late so the
            # PE never fences on the normalize chain)
            for ms in range(4):
                sl = slice(qc * 512 + ms * P, qc * 512 + (ms + 1) * P)
                for dc in range(4):
                    pso = opp.tile([P, 512], f32, tag="op", name="pso")
                    for kc in range(2):
                        nc.tensor.matmul(
                            pso, OT[:, kc, sl], wo[:, kc, dc * 512 : (dc + 1) * 512],
                            start=(kc == 0), stop=(kc == 1),
                        )
                    ov = ovp.tile([P, 512], bf, tag="ov", name="ov")
                    nc.vector.tensor_copy(ov[:], pso[:])
                    nc.sync.dma_start(io["out"][sl, dc * 512 : (dc + 1) * 512], ov[:])

        for qc in range(4):
            qs = slice(qc * 512, (qc + 1) * 512)
            nkb = 4 * qc + 4
            for pair in range(2):
                Q = QT[pair]
                po = [op_.tile([HD + 1, 512], f32, tag="o", name="po")
                      for _ in range(2)]

                def score_exp(kb):
                    ps2 = sp.tile([P, 2, 512], f32, tag="s", name="ps2")
                    kbs = slice(kb * P, (kb + 1) * P)
                    nc.tensor.matmul(ps2[:, 0, :], KT[0:HD, kbs],
                                     Q[0:HD, qs], start=True, stop=True,
                                     tile_position=(0, 0))
                    nc.tensor.matmul(ps2[:, 1, :], KT[HD:P, kbs],
                                     Q[HD:P, qs], start=True, stop=True,
                                     tile_position=(HD, 0))
                    es2 = ep.tile([P, 2, 512], bf, tag="e", name="es2")
                    nc.scalar.activation(es2[:], ps2[:], Exp, scale=SCALE)
                    o = kb - 4 * qc
                    if o >= 0:
                        mz = maskp[:, :, 512 - o * P : 1024 - o * P]
                        nc.vector.tensor_mul(es2[:], es2[:], mz)
                    return es2

                def pv_acc(kb, es2):
                    st = (kb == 0)
                    sp_ = (kb == nkb - 1)
                    for j in range(2):
                        nc.tensor.matmul(po[j], V[:, kb, :], es2[:, j, :],
                                         start=st, stop=sp_)

                prev = score_exp(0)
                for kb in range(1, nkb):
                    cur = score_exp(kb)
                    pv_acc(kb - 1, prev)
                    prev = cur
                pv_acc(nkb - 1, prev)

                # normalize: row HD of po holds the softmax denominators
                for j in range(2):
                    den = rcp.tile([1, 512], f32, tag="den", name="den")
                    nc.vector.tensor_copy(den, po[j][HD : HD + 1, :])
                    rec = rcp.tile([1, 512], f32, tag="rec", name="rec")
                    nc.vector.reciprocal_approx_fast(rec, den)
                    bcs = bcsp.tile([HD, 512], f32, tag="bcs", name="bcs")
                    nc.gpsimd.partition_broadcast(bcs, rec)
                    if j == 0:
                        nc.vector.tensor_mul(OT[0:HD, pair, qs],
                                             po[j][0:HD, :], bcs)
                    else:
                        stg = stgp.tile([HD, 512], bf, tag="stg", name="stg")
                        nc.vector.tensor_mul(stg, po[j][0:HD, :], bcs)
                        nc.sync.dma_start(OT[HD:P, pair, qs], stg[:])

            if qc > 0:
                out_proj(qc - 1)
        out_proj(3)


def _prep_core_inputs(i, x, cos, sin, g_q, g_k, Wq, Wk, Wv, Wo):
    c0 = i * 4 * HD
    k0 = i * HD

    def b(a):
        return np.ascontiguousarray(a.astype(BF16))

    x2d = x.reshape(S, D)
    # xt4[p, sc, kc, j] = x[sc*512+j, kc*128+p]
    xt4 = b(x2d.T.reshape(16, P, 4, 512).transpose(1, 2, 0, 3))
    wqa = b(Wq[:, c0 : c0 + P].reshape(16, P, P).transpose(1, 0, 2))
    wqb = b(Wq[:, c0 + P : c0 + 2 * P].reshape(16, P, P).transpose(1, 0, 2))
    wk = b(Wk[:, k0 : k0 + HD].reshape(16, P, HD).transpose(1, 0, 2))
    wv = b(Wv[:, k0 : k0 + HD].reshape(16, P, HD).transpose(1, 0, 2))
    wo = b(Wo[c0 : c0 + 2 * P, :].reshape(2, P, D).transpose(1, 0, 2))
    cosT = cos.T.astype(np.float32)  # [32, S]
    sinT = sin.T.astype(np.float32)
    cos4 = b(np.tile(cosT, (4, 1)))
    sin4s = b(np.concatenate([-sinT, sinT, -sinT, sinT], axis=0))
    gq2 = np.tile(g_q, 2)[:, None].astype(np.float32)
    gk = g_k[:, None].astype(np.float32)
    tri = np.triu(np.ones((P, P), dtype=np.float32))  # [k within blk, q within blk]
    mask0 = np.concatenate([tri, np.ones((P, 384), dtype=np.float32)], axis=1)
    maskz = np.concatenate([np.zeros((P, 512), dtype=np.float32), mask0], axis=1)
    maskp = b(np.stack([maskz, maskz], axis=1))  # [128, 2, 1024]
    ones65 = np.zeros((P, HD + 1), dtype=np.float32)
    ones65[:HD, 0] = 1.0
    ones65[HD:, HD] = 1.0
    r64 = np.roll(np.eye(HD, dtype=np.float32), 32, axis=0)
    rot2 = np.zeros((P, P), dtype=np.float32)
    rot2[:HD, :HD] = r64
    rot2[HD:, HD:] = r64
    return {
        "xt4": xt4,
        "wqa": wqa, "wqb": wqb, "wk": wk, "wv": wv, "wo": wo,
        "cos4": cos4, "sin4s": sin4s,
        "gq2": gq2, "gk": gk, "maskp": maskp,
        "ones65": b(ones65),
        "rot2": b(rot2),
    }


def kernel(x, cos, sin, g_q, g_k, Wq, Wk, Wv, Wo):
    global LAST_RESULTS
    from concourse.bass_utils import run_bass_kernel_spmd

    if "nc" not in _CACHE:
        _CACHE["nc"] = _build_nc()
    nc = _CACHE["nc"]

    args = [np.asarray(a, dtype=np.float32) for a in
            (x, cos, sin, g_q, g_k, Wq, Wk, Wv, Wo)]
    in_maps = [_prep_core_inputs(i, *args) for i in range(N_CORES)]
    trace = bool(os.environ.get("BASS_TRACE"))
    res = run_bass_kernel_spmd(nc, in_maps, list(range(N_CORES)), trace=trace)
    LAST_RESULTS = res
    out = np.zeros((S, D), dtype=np.float32)
    for r in res.results:
        out += np.asarray(r["out"], dtype=np.float32)
    return out.reshape(1, S, D)
